# revision 24
# baseline (speedup 1.0000x reference)
"""BD3LM block-diffusion decoder layer on 8 trn2 NeuronCores.

Sharding: core = 2*b + g  (b = batch 0..3, g = head-group 0..1, 8 heads each).
Each core: QKV projections for its batch/head-group, sparse BD3LM attention
(only ~80 of 256 score tiles per head), O-projection against its Wo row-slice.
Host: sums the two group partials per batch and adds the (bv @ Wo + bo)
correction (softmax rows sum to 1, so the v-bias contributes exactly bv @ Wo).

Layouts on device (per core):
  qT/kT  [d_head_group=512, T=2048]  stored [128, 4, 2048]  (d on partitions)
  v      [T, 512] stored [128, 16, 8*65] with a per-head ones column -> the
         ctx matmul accumulates softmax denominators for free (row 64).
  scores computed transposed [k_tile=128, q_span] so softmax reduction is a
         PE matmul instead of a partition reduction; exp on ACT without
         max-subtraction (scores are ~N(0,1), bias-free overflow impossible);
         only 3 distinct 128x128 binary mask tiles (strict/incl/diag).
"""

import numpy as np

import concourse.bass as bass
import concourse.mybir as mybir
import concourse.tile as tile
from concourse import bacc
from concourse.bass_utils import run_bass_kernel_spmd

F32 = mybir.dt.float32
F32R = mybir.dt.float32r
BF16 = mybir.dt.bfloat16
Act = mybir.ActivationFunctionType

B, T, D = 4, 2048, 1024
H, HD = 16, 64
L = T // 2           # 1024, length of each of [xt | x0]
BS = 4               # block size
G = 2                # head groups (cores per batch)
DG = D // G          # 512 channels per group
HG = H // G          # 8 heads per core
P = 128
NT = L // P          # 8 key/query tiles per half
SLAB = 256           # projection t-slab width
KC = D // P          # 8 contraction chunks
DT4 = DG // P        # 4 output-partition tiles for qT/kT

# matmul dtype per family: float32 (exact, 4 cyc/row), float32r (~2.7e-4
# end-to-end, 1 cyc/row at N>=256 but 4 cyc/row below), bf16 (1 cyc/row at
# any width, ~0.4% rounding). Attention runs bf16: its matmuls include many
# <256-wide outputs (diag tiles, odd chunks) where f32r pays 4x.
PROJ_DT = F32R
ATTN_DT = BF16
OPROJ_DT = F32R
BCAST_DT = F32R

REPEAT = 1  # loop whole computation inside the NEFF (timing experiments only)
REPEAT_PHASES = "ABC"  # which phases the extra REPEAT-1 iterations run
DBG = False

_CACHE = {}


def _chunks512(a0, a1):
    """Split [a0, a1) at multiples of 512 (PSUM bank boundaries)."""
    out = []
    while a0 < a1:
        b1 = min(a1, (a0 // 512 + 1) * 512)
        out.append((a0, b1))
        a0 = b1
    return out


def _mm(ap, dt):
    return ap.bitcast(dt) if dt != F32 else ap


def _build():
    import concourse.tile_utils as tile_utils

    tile_utils.max_sbuf_usage = 204 * 1024  # trn2 has 208KB/partition usable

    nc = bacc.Bacc("TRN2", target_bir_lowering=False, debug=False, num_devices=8)
    dbg = {}
    if DBG:
        for nm, shp in (
            ("dbg_qT", [P, DT4, T]),
            ("dbg_kT", [P, DT4, T]),
            ("dbg_v", [P, T // P, HG * (HD + 1)]),
            ("dbg_ctxT", [P, DT4, T]),
            ("dbg_nd", [16, HD + 1, L]),
            ("dbg_at", [P, L]),
        ):
            dbg[nm] = nc.dram_tensor(nm, shp, F32, kind="ExternalOutput").ap()

    xT = nc.dram_tensor("xT", [D, T], F32, kind="ExternalInput").ap()
    wq = nc.dram_tensor("wq", [D, DG], F32, kind="ExternalInput").ap()
    wk = nc.dram_tensor("wk", [D, DG], F32, kind="ExternalInput").ap()
    wv = nc.dram_tensor("wv", [D, DG], F32, kind="ExternalInput").ap()
    wo = nc.dram_tensor("wo", [DG, D], F32, kind="ExternalInput").ap()
    bqs = nc.dram_tensor("bqs", [DG], F32, kind="ExternalInput").ap()
    bks = nc.dram_tensor("bks", [DG], F32, kind="ExternalInput").ap()
    msk = nc.dram_tensor("msk", [4, P, P], ATTN_DT, kind="ExternalInput").ap()
    out = nc.dram_tensor("out", [T, D], F32, kind="ExternalOutput").ap()

    views = dict(
        xT_v=xT.rearrange("(kc p) t -> p kc t", p=P),    # [128, 8, 2048]
        wq_v=wq.rearrange("(kc p) m -> p kc m", p=P),    # [128, 8, 512]
        wk_v=wk.rearrange("(kc p) m -> p kc m", p=P),
        wv_v=wv.rearrange("(kc p) m -> p kc m", p=P),
        wo_v=wo.rearrange("(cc p) n -> p cc n", p=P),    # [128, 4, 1024]
        msk=msk,
        out=out,
    )

    with tile.TileContext(nc) as tc:
        with tc.tile_pool(name="persist", bufs=1) as pers:
            st = dict(
                qT_sb=pers.tile([P, DT4, T], ATTN_DT, name="qT_sb"),
                kT_sb=pers.tile([P, DT4, T], ATTN_DT, name="kT_sb"),
                v_sb=pers.tile([P, T // P, HG * (HD + 1)], ATTN_DT, name="v_sb"),
                bq_sb=pers.tile([P, DT4], F32, name="bq_sb"),
                bk_sb=pers.tile([P, DT4], F32, name="bk_sb"),
            )
            nc.sync.dma_start(st["bq_sb"], bqs.rearrange("(c p) -> p c", p=P))
            nc.sync.dma_start(st["bk_sb"], bks.rearrange("(c p) -> p c", p=P))
            # ones columns for the softmax denominators
            ones_c = pers.tile([P, 1], F32, name="ones_c")
            nc.vector.memset(ones_c, 1.0)
            ones_v = st["v_sb"].rearrange("p t (h c) -> p (t h) c", c=HD + 1)[
                :, :, HD : HD + 1
            ]
            nc.vector.tensor_copy(
                ones_v, ones_c[:, 0:1, None].to_broadcast(tuple(ones_v.shape))
            )
            st["ones_c"] = ones_c

            _phases(nc, tc, dbg, st, views)
            for _rep in range(REPEAT - 1):
                _phases(nc, tc, dbg, st, views, phases=REPEAT_PHASES)

    nc.compile()
    return nc


def _phases(nc, tc, dbg, st, views, phases="ABC"):
    if "A" in phases:
        _phase_a(nc, tc, dbg, st, views)
    if "B" in phases or "C" in phases:
        with tc.tile_pool(name="bcpool", bufs=1) as bcp:
            st2 = dict(
                st,
                ctxT_sb=bcp.tile([P, DT4, T], F32, name="ctxT_sb"),
                wo_sb=bcp.tile([P, DT4, D], F32, name="wo_sb"),
            )
            if "B" in phases:
                _phase_b(nc, tc, dbg, st2, views)
            if "C" in phases:
                _phase_c(nc, tc, dbg, st2, views)


def _phase_a(nc, tc, dbg, st, views):
    qT_sb, kT_sb, v_sb = st["qT_sb"], st["kT_sb"], st["v_sb"]
    xT_v = views["xT_v"]

    # ---------------- Phase A: QKV projections (one x stream) ----------------
    with (
        tc.tile_pool(name="wpool", bufs=1) as wpool,
        tc.tile_pool(name="xpool", bufs=3) as xpool,
        tc.tile_pool(name="ppsum", bufs=4, space="PSUM") as ppsum,
        tc.tile_pool(name="vpsum", bufs=4, space="PSUM") as vpsum,
    ):
        wq_sb = wpool.tile([P, KC, DG], F32, name="wq_sb")
        wk_sb = wpool.tile([P, KC, DG], F32, name="wk_sb")
        wv_sb = wpool.tile([P, KC, DG], F32, name="wv_sb")
        x_tiles = []
        for s in range(T // 512):
            x_sb = xpool.tile([P, KC, 512], F32, tag="x", name=f"x{s}")
            if s < 2:  # prefetch depth 2; later slabs DMA'd in the loop
                nc.sync.dma_start(
                    _mm(x_sb, PROJ_DT),
                    _mm(xT_v[:, :, 512 * s : 512 * (s + 1)], PROJ_DT),
                )
            x_tiles.append(x_sb)
        # wq split per column-tile: the d4=0 matmuls only wait on 0.5MB of wq
        for d4 in range(DT4):
            nc.sync.dma_start(
                _mm(wq_sb[:, :, P * d4 : P * (d4 + 1)], PROJ_DT),
                _mm(views["wq_v"][:, :, P * d4 : P * (d4 + 1)], PROJ_DT),
            )
        nc.sync.dma_start(_mm(wk_sb, PROJ_DT), _mm(views["wk_v"], PROJ_DT))
        nc.sync.dma_start(_mm(wv_sb, PROJ_DT), _mm(views["wv_v"], PROJ_DT))
        for s in range(T // 512):
            x_sb = x_tiles[s]
            if s >= 2:
                nc.sync.dma_start(
                    _mm(x_sb, PROJ_DT),
                    _mm(xT_v[:, :, 512 * s : 512 * (s + 1)], PROJ_DT),
                )
            for w_sb, b_key, dst, scale in (
                (wq_sb, "bq_sb", qT_sb, HD ** -0.5),
                (wk_sb, "bk_sb", kT_sb, 1.0),
            ):
                for d4 in range(DT4):
                    ps = ppsum.tile([P, 512], F32, tag="pp", name=f"pp{s}_{d4}")
                    for kc in range(KC):
                        nc.tensor.matmul(
                            ps,
                            _mm(w_sb[:, kc, P * d4 : P * (d4 + 1)], PROJ_DT),
                            _mm(x_sb[:, kc, :], PROJ_DT),
                            start=(kc == 0),
                            stop=(kc == KC - 1),
                        )
                    nc.scalar.activation(
                        dst[:, d4, 512 * s : 512 * (s + 1)],
                        ps,
                        Act.Identity,
                        bias=st[b_key][:, d4 : d4 + 1],
                        scale=scale,
                    )
            for t2 in range(4):
                tt = 4 * s + t2
                ps = vpsum.tile([P, DG], F32, tag="ppv", name=f"ppv{tt}")
                for kc in range(KC):
                    nc.tensor.matmul(
                        ps,
                        _mm(x_sb[:, kc, P * t2 : P * (t2 + 1)], PROJ_DT),
                        _mm(wv_sb[:, kc, :], PROJ_DT),
                        start=(kc == 0),
                        stop=(kc == KC - 1),
                    )
                nc.vector.tensor_copy(
                    v_sb[:, tt].rearrange("p (h c) -> p h c", c=HD + 1)[:, :, :HD],
                    ps.rearrange("p (h c) -> p h c", c=HD),
                )

def _phase_b(nc, tc, dbg, st, views):
    qT_sb, kT_sb, v_sb = st["qT_sb"], st["kT_sb"], st["v_sb"]
    ctxT_sb, wo_sb = st["ctxT_sb"], st["wo_sb"]
    wo_v, msk = views["wo_v"], views["msk"]

    # ---------------- Phase B: sparse attention ----------------
    # Software-pipelined: per head-half, ALL score(+mask-add)+exp chunks are
    # emitted first, then all ctx matmuls. The PE stream never waits on the
    # ACT exp chain (exp(c) completes while PE runs scores c+1..). Masks are
    # additive, applied by PE matmuls (identity stationary) accumulating into
    # the score PSUM group — no DVE op on the critical chain. Normalization
    # of head-half X is emitted after the scores of X+1 so its PE broadcast
    # never stalls the queue.
    with (
        tc.tile_pool(name="apool", bufs=1) as apool,
        tc.tile_pool(name="tmppool", bufs=2) as tmppool,
        tc.tile_pool(name="atpool", bufs=24) as atpool,
        tc.tile_pool(name="spsum", bufs=3, space="PSUM") as spsum,
        tc.tile_pool(name="cpsum", bufs=2, space="PSUM") as cpsum,
    ):
        nc.sync.dma_start(_mm(wo_sb, OPROJ_DT), _mm(wo_v, OPROJ_DT))
        m_strict = apool.tile([P, P], ATTN_DT, name="m_strict")
        m_incl = apool.tile([P, P], ATTN_DT, name="m_incl")
        m_diag = apool.tile([P, P], ATTN_DT, name="m_diag")
        ident = apool.tile([P, P], ATTN_DT, name="ident")
        nc.sync.dma_start(m_strict, msk[0])
        nc.sync.dma_start(m_incl, msk[1])
        nc.sync.dma_start(m_diag, msk[2])
        nc.sync.dma_start(ident, msk[3])
        ones_t = apool.tile([P, HD], F32, name="ones_t")  # row 64: K=1 bcast lhsT
        if BCAST_DT == F32:
            nc.vector.memset(ones_t, 1.0)
        else:
            nc.vector.tensor_copy(
                _mm(ones_t, BCAST_DT),
                st["ones_c"][:, 0:1].to_broadcast((P, HD)),
            )

        def scores_half(h, half):
            """Emit score+mask-add+exp for all chunks; return ctx work list."""
            c, p0 = h // 2, HD * (h % 2)
            qh = qT_sb[p0 : p0 + HD, c, :]   # [64, 2048]
            kh = kT_sb[p0 : p0 + HD, c, :]
            mask = m_strict if half == 0 else m_incl
            work = []
            for j in range(NT):
                kv = kh[:, L + P * j : L + P * (j + 1)]                  # [64, 128]
                vj = v_sb[:, NT + j, (HD + 1) * h : (HD + 1) * (h + 1)]  # [128, 65]
                for a0, a1 in _chunks512(P * j, L):
                    n = a1 - a0
                    masked = a0 == P * j
                    sc = spsum.tile(
                        [P, 512], F32, tag="sc", name=f"sc{h}_{j}_{half}_{a0}"
                    )[:, :n]
                    nc.tensor.matmul(
                        sc,
                        kv,
                        qh[:, L * half + a0 : L * half + a1],
                        start=True,
                        stop=not masked,
                    )
                    if masked:
                        nc.tensor.matmul(
                            sc[:, :P], ident, mask, start=False, stop=True
                        )
                    at = atpool.tile(
                        [P, 512], ATTN_DT, tag="at", name=f"at{h}_{j}_{half}_{a0}"
                    )[:, :n]
                    nc.scalar.activation(at, sc, Act.Exp)
                    # x0 half: stop on the last j touching this bank
                    last = half == 1 and (
                        (a1 <= 512 and j == 3) or (a0 >= 512 and j == NT - 1)
                    )
                    work.append((vj, at, a0, a1, j == 0, last))
            if half == 0:
                # xt-xt block-diagonal tiles
                for i in range(NT):
                    scd = spsum.tile(
                        [P, 512], F32, tag="sc", name=f"scd{h}_{i}"
                    )[:, :P]
                    nc.tensor.matmul(
                        scd,
                        kh[:, P * i : P * (i + 1)],
                        qh[:, P * i : P * (i + 1)],
                        start=True,
                        stop=False,
                    )
                    nc.tensor.matmul(scd, ident, m_diag, start=False, stop=True)
                    atd = atpool.tile(
                        [P, 512], ATTN_DT, tag="at", name=f"atd{h}_{i}"
                    )[:, :P]
                    nc.scalar.activation(atd, scd, Act.Exp)
                    vd = v_sb[:, i, (HD + 1) * h : (HD + 1) * (h + 1)]
                    work.append((vd, atd, P * i, P * (i + 1), False, i == 3 or i == NT - 1))
            return work

        def normalize(h, half, ctx):
            # ctxT = ctx[:64] * (1 / denom), denom = row 64
            c = h // 2
            recip = tmppool.tile([P, L], F32, tag="recip", name=f"rc{h}_{half}")
            with nc.allow_low_precision(reason="deliberate f32r rounding"):
                nc.vector.reciprocal(
                    _mm(recip[HD : HD + 1, :], BCAST_DT),
                    ctx[HD : HD + 1, :],
                )
            rb = tmppool.tile([HD, L], F32, tag="rb", bufs=3, name=f"rb{h}_{half}")
            # PE broadcast: ones[1,64].T @ recip[1,n] -> [64, n]
            for c0 in range(0, L, 512):
                bc = spsum.tile(
                    [P, 512], F32, tag="bc", bufs=1, name=f"bc{h}_{half}_{c0}"
                )[:HD, :]
                nc.tensor.matmul(
                    bc,
                    _mm(ones_t[HD : HD + 1, :], BCAST_DT),
                    _mm(recip[HD : HD + 1, c0 : c0 + 512], BCAST_DT),
                    start=True,
                    stop=True,
                )
                nc.vector.tensor_copy(rb[:, c0 : c0 + 512], bc)
            if h % 2 == 0:
                nc.vector.tensor_mul(
                    _mm(ctxT_sb[:HD, c, L * half : L * (half + 1)], OPROJ_DT),
                    ctx[:HD, :],
                    rb,
                )
            else:
                cs = tmppool.tile([HD, L], F32, tag="cs", bufs=3, name=f"cs{h}_{half}")
                nc.vector.tensor_mul(_mm(cs, OPROJ_DT), ctx[:HD, :], rb)
                nc.sync.dma_start(
                    _mm(ctxT_sb[HD : 2 * HD, c, L * half : L * (half + 1)], OPROJ_DT),
                    _mm(cs, OPROJ_DT),
                )

        prev = None
        for h in range(HG):
            for half in range(2):
                work = scores_half(h, half)
                if prev is not None:
                    normalize(*prev)
                ctx = cpsum.tile([HD + 1, L], F32, tag="ctx", name=f"ctx{h}_{half}")
                for vj, at, a0, a1, start, stop in work:
                    nc.tensor.matmul(ctx[:, a0:a1], vj, at, start=start, stop=stop)
                prev = (h, half, ctx)
        normalize(*prev)

        if DBG:
            nc.sync.dma_start(dbg["dbg_qT"], qT_sb)
            nc.sync.dma_start(dbg["dbg_kT"], kT_sb)
            nc.sync.dma_start(dbg["dbg_v"], v_sb)
            nc.sync.dma_start(dbg["dbg_ctxT"], ctxT_sb)


def _phase_c(nc, tc, dbg, st, views):
    ctxT_sb, wo_sb = st["ctxT_sb"], st["wo_sb"]
    out = views["out"]

    # ---------------- Phase C: O-projection ----------------
    with (
        tc.tile_pool(name="opsum", bufs=6, space="PSUM") as opsum,
        tc.tile_pool(name="osbpool", bufs=6) as osbpool,
    ):
        for tt in range(T // P):
            for nk in range(2):
                ops = opsum.tile([P, 512], F32, tag="op", name=f"op{tt}_{nk}")
                for cc in range(DT4):
                    nc.tensor.matmul(
                        ops,
                        _mm(ctxT_sb[:, cc, P * tt : P * (tt + 1)], OPROJ_DT),
                        _mm(wo_sb[:, cc, 512 * nk : 512 * (nk + 1)], OPROJ_DT),
                        start=(cc == 0),
                        stop=(cc == DT4 - 1),
                    )
                osb = osbpool.tile([P, 512], F32, tag="osb", name=f"osb{tt}_{nk}")
                nc.vector.tensor_copy(osb, ops)
                nc.sync.dma_start(
                    out[P * tt : P * (tt + 1), 512 * nk : 512 * (nk + 1)], osb
                )


def _masks():
    """Additive masks (0 = attend, -30 = masked; exp(-30)~9e-14) plus an
    identity tile used as the stationary operand of mask-add matmuls."""
    q = np.arange(P)[None, :] // BS
    k = np.arange(P)[:, None] // BS
    m = np.zeros((4, P, P), np.float32)
    m[0] = np.where(q > k, 0.0, -30.0)   # strict (xt q vs x0 k, same tile)
    m[1] = np.where(q >= k, 0.0, -30.0)  # incl (x0 q vs x0 k, same tile)
    m[2] = np.where(q == k, 0.0, -30.0)  # diag (xt q vs xt k, same tile)
    m[3] = np.eye(P, dtype=np.float32)
    return m


def kernel(x, Wq, bq, Wk, bk, Wv, bv, Wo, bo, block_size=4, **_):
    x = np.asarray(x, np.float32)
    Wq, bq = np.asarray(Wq, np.float32), np.asarray(bq, np.float32)
    Wk, bk = np.asarray(Wk, np.float32), np.asarray(bk, np.float32)
    Wv, bv = np.asarray(Wv, np.float32), np.asarray(bv, np.float32)
    Wo, bo = np.asarray(Wo, np.float32), np.asarray(bo, np.float32)

    if "nc" not in _CACHE:
        _CACHE["nc"] = _build()
    nc = _CACHE["nc"]

    masks = _masks().astype(mybir.dt.np(ATTN_DT))
    scale = HD ** -0.5
    in_maps = []
    for core in range(8):
        b, g = core // 2, core % 2
        cols = slice(DG * g, DG * (g + 1))
        in_maps.append(
            {
                "xT": np.ascontiguousarray(x[b].T),
                "wq": np.ascontiguousarray(Wq[:, cols]),
                "wk": np.ascontiguousarray(Wk[:, cols]),
                "wv": np.ascontiguousarray(Wv[:, cols]),
                "wo": np.ascontiguousarray(Wo[cols, :]),
                "bqs": np.ascontiguousarray(bq[cols]) * np.float32(scale),
                "bks": np.ascontiguousarray(bk[cols]),
                "msk": masks,
            }
        )

    _CACHE["last_in_maps"] = in_maps
    last_err = None
    for _attempt in range(6):
        try:
            res = run_bass_kernel_spmd(nc, in_maps, core_ids=list(range(8)), trace=False)
            break
        except Exception as e:  # transient NRT device flakes
            last_err = e
            msg = str(e)
            if "UNRECOVERABLE" not in msg and "UNAVAILABLE" not in msg:
                raise
            import time as _time

            import jax as _jax

            _time.sleep(5 * (_attempt + 1))
            try:
                _jax.clear_backends()
            except Exception:
                pass
    else:
        raise last_err

    corr = (bv @ Wo + bo).astype(np.float32)  # softmax rows sum to 1
    out = np.empty((B, T, D), np.float32)
    for b in range(B):
        out[b] = res.results[2 * b]["out"] + res.results[2 * b + 1]["out"] + corr
    return out


if __name__ == "__main__":
    rng = np.random.default_rng(0)
    inputs = {
        "x": rng.standard_normal((B, T, D)).astype(np.float32),
        "Wq": (rng.standard_normal((D, D)) / 32).astype(np.float32),
        "bq": np.zeros(D, np.float32),
        "Wk": (rng.standard_normal((D, D)) / 32).astype(np.float32),
        "bk": np.zeros(D, np.float32),
        "Wv": (rng.standard_normal((D, D)) / 32).astype(np.float32),
        "bv": np.zeros(D, np.float32),
        "Wo": (rng.standard_normal((D, D)) / 32).astype(np.float32),
        "bo": np.zeros(D, np.float32),
    }
    o = kernel(**inputs)
    print("ran", o.shape, o.dtype, float(np.abs(o).max()))



# revision 33
# speedup vs baseline: 1.4391x; 1.4391x over previous
"""BD3LM block-diffusion decoder layer on 8 trn2 NeuronCores.

Sharding: core = 2*b + g  (b = batch 0..3, g = head-group 0..1, 8 heads each).
Each core: QKV projections for its batch/head-group, sparse BD3LM attention
(only ~80 of 256 score tiles per head), O-projection against its Wo row-slice.
Host: sums the two group partials per batch and adds the (bv @ Wo + bo)
correction (softmax rows sum to 1, so the v-bias contributes exactly bv @ Wo).

Layouts on device (per core):
  qT/kT  [d_head_group=512, T=2048]  stored [128, 4, 2048]  (d on partitions)
  v      [T, 512] stored [128, 16, 8*65] with a per-head ones column -> the
         ctx matmul accumulates softmax denominators for free (row 64).
  scores computed transposed [k_tile=128, q_span] so softmax reduction is a
         PE matmul instead of a partition reduction; exp on ACT without
         max-subtraction (scores are ~N(0,1), bias-free overflow impossible);
         only 3 distinct 128x128 binary mask tiles (strict/incl/diag).
"""

import numpy as np

import concourse.bass as bass
import concourse.mybir as mybir
import concourse.tile as tile
from concourse import bacc
from concourse.bass_utils import run_bass_kernel_spmd

F32 = mybir.dt.float32
F32R = mybir.dt.float32r
BF16 = mybir.dt.bfloat16
Act = mybir.ActivationFunctionType

B, T, D = 4, 2048, 1024
H, HD = 16, 64
L = T // 2           # 1024, length of each of [xt | x0]
BS = 4               # block size
G = 2                # head groups (cores per batch)
DG = D // G          # 512 channels per group
HG = H // G          # 8 heads per core
P = 128
NT = L // P          # 8 key/query tiles per half
SLAB = 256           # projection t-slab width
KC = D // P          # 8 contraction chunks
DT4 = DG // P        # 4 output-partition tiles for qT/kT
VW = 2 * HD          # v columns per head: 64 channels + 64 ones (denom rows)

# matmul dtype per family: float32 (exact, 4 cyc/row), float32r (~2.7e-4
# end-to-end, 1 cyc/row at N>=256 but 4 cyc/row below), bf16 (1 cyc/row at
# any width, ~0.4% rounding). Attention runs bf16: its matmuls include many
# <256-wide outputs (diag tiles, odd chunks) where f32r pays 4x.
PROJ_DT = F32R
ATTN_DT = BF16
OPROJ_DT = F32R
BCAST_DT = F32R

REPEAT = 1  # loop whole computation inside the NEFF (timing experiments only)
REPEAT_PHASES = "ABC"  # which phases the extra REPEAT-1 iterations run
DBG = False

_CACHE = {}


def _chunks512(a0, a1):
    """Split [a0, a1) at multiples of 512 (PSUM bank boundaries)."""
    out = []
    while a0 < a1:
        b1 = min(a1, (a0 // 512 + 1) * 512)
        out.append((a0, b1))
        a0 = b1
    return out


def _mm(ap, dt):
    return ap.bitcast(dt) if dt != F32 else ap


def _build():
    import concourse.tile_utils as tile_utils

    tile_utils.max_sbuf_usage = 204 * 1024  # trn2 has 208KB/partition usable

    nc = bacc.Bacc("TRN2", target_bir_lowering=False, debug=False, num_devices=8)
    dbg = {}
    if DBG:
        for nm, shp in (
            ("dbg_qT", [P, DT4, T]),
            ("dbg_kT", [P, DT4, T]),
            ("dbg_v", [P, T // P, HG * (HD + 1)]),
            ("dbg_ctxT", [P, DT4, T]),
            ("dbg_nd", [16, HD + 1, L]),
            ("dbg_at", [P, L]),
        ):
            dbg[nm] = nc.dram_tensor(nm, shp, F32, kind="ExternalOutput").ap()

    xT = nc.dram_tensor("xT", [D, T], F32, kind="ExternalInput").ap()
    wq = nc.dram_tensor("wq", [D, DG], F32, kind="ExternalInput").ap()
    wk = nc.dram_tensor("wk", [D, DG], F32, kind="ExternalInput").ap()
    wv = nc.dram_tensor("wv", [D, DG], F32, kind="ExternalInput").ap()
    wo = nc.dram_tensor("wo", [DG, D], F32, kind="ExternalInput").ap()
    bqs = nc.dram_tensor("bqs", [DG], F32, kind="ExternalInput").ap()
    bks = nc.dram_tensor("bks", [DG], F32, kind="ExternalInput").ap()
    msk = nc.dram_tensor("msk", [2, P, P], ATTN_DT, kind="ExternalInput").ap()
    mskd = nc.dram_tensor("mskd", [P, 4 * P], ATTN_DT, kind="ExternalInput").ap()
    out = nc.dram_tensor("out", [T, D], F32, kind="ExternalOutput").ap()

    views = dict(
        xT_v=xT.rearrange("(kc p) t -> p kc t", p=P),    # [128, 8, 2048]
        wq_v=wq.rearrange("(kc p) m -> p kc m", p=P),    # [128, 8, 512]
        wk_v=wk.rearrange("(kc p) m -> p kc m", p=P),
        wv_v=wv.rearrange("(kc p) m -> p kc m", p=P),
        wo_v=wo.rearrange("(cc p) n -> p cc n", p=P),    # [128, 4, 1024]
        msk=msk,
        mskd=mskd,
        out=out,
    )

    with tile.TileContext(nc) as tc:
        with tc.tile_pool(name="persist", bufs=1) as pers:
            st = dict(
                qT_sb=pers.tile([P, DT4, T], ATTN_DT, name="qT_sb"),
                kT_sb=pers.tile([P, DT4, T], ATTN_DT, name="kT_sb"),
                v_sb=pers.tile([P, T // P, HG * VW], ATTN_DT, name="v_sb"),
                bq_sb=pers.tile([P, DT4], F32, name="bq_sb"),
                bk_sb=pers.tile([P, DT4], F32, name="bk_sb"),
            )
            nc.sync.dma_start(st["bq_sb"], bqs.rearrange("(c p) -> p c", p=P))
            nc.sync.dma_start(st["bk_sb"], bks.rearrange("(c p) -> p c", p=P))
            # ones block (cols 64:128 of each head's v): the ctx matmul then
            # emits softmax denominators replicated on psum rows 64:128
            ones_c = pers.tile([P, 1], F32, name="ones_c")
            nc.vector.memset(ones_c, 1.0)
            ones_v = st["v_sb"].rearrange("p t (h w) -> p (t h) w", w=VW)[
                :, :, HD:
            ]
            nc.vector.tensor_copy(
                ones_v, ones_c[:, 0:1, None].to_broadcast(tuple(ones_v.shape))
            )
            st["ones_c"] = ones_c

            _phases(nc, tc, dbg, st, views)
            for _rep in range(REPEAT - 1):
                _phases(nc, tc, dbg, st, views, phases=REPEAT_PHASES)

    nc.compile()
    return nc


def _phases(nc, tc, dbg, st, views, phases="ABC"):
    if "A" in phases:
        _phase_a(nc, tc, dbg, st, views)
    if "B" in phases or "C" in phases:
        with tc.tile_pool(name="bcpool", bufs=1) as bcp:
            st2 = dict(
                st,
                ctxT_sb=bcp.tile([P, DT4, T], F32, name="ctxT_sb"),
                wo_sb=bcp.tile([P, DT4, D], F32, name="wo_sb"),
            )
            if "B" in phases:
                _phase_b(nc, tc, dbg, st2, views)
            if "C" in phases:
                _phase_c(nc, tc, dbg, st2, views)


def _phase_a(nc, tc, dbg, st, views):
    qT_sb, kT_sb, v_sb = st["qT_sb"], st["kT_sb"], st["v_sb"]
    xT_v = views["xT_v"]

    # ---------------- Phase A: QKV projections (one x stream) ----------------
    with (
        tc.tile_pool(name="wpool", bufs=1) as wpool,
        tc.tile_pool(name="xpool", bufs=3) as xpool,
        tc.tile_pool(name="ppsum", bufs=4, space="PSUM") as ppsum,
        tc.tile_pool(name="vpsum", bufs=4, space="PSUM") as vpsum,
    ):
        wq_sb = wpool.tile([P, KC, DG], F32, name="wq_sb")
        wk_sb = wpool.tile([P, KC, DG], F32, name="wk_sb")
        wv_sb = wpool.tile([P, KC, DG], F32, name="wv_sb")
        x_tiles = []
        for s in range(T // 512):
            x_sb = xpool.tile([P, KC, 512], F32, tag="x", name=f"x{s}")
            if s < 2:  # prefetch depth 2; later slabs DMA'd in the loop
                nc.sync.dma_start(
                    _mm(x_sb, PROJ_DT),
                    _mm(xT_v[:, :, 512 * s : 512 * (s + 1)], PROJ_DT),
                )
            x_tiles.append(x_sb)
        # wq split per column-tile: the d4=0 matmuls only wait on 0.5MB of wq
        for d4 in range(DT4):
            nc.sync.dma_start(
                _mm(wq_sb[:, :, P * d4 : P * (d4 + 1)], PROJ_DT),
                _mm(views["wq_v"][:, :, P * d4 : P * (d4 + 1)], PROJ_DT),
            )
        nc.sync.dma_start(_mm(wk_sb, PROJ_DT), _mm(views["wk_v"], PROJ_DT))
        nc.sync.dma_start(_mm(wv_sb, PROJ_DT), _mm(views["wv_v"], PROJ_DT))
        for s in range(T // 512):
            x_sb = x_tiles[s]
            if s >= 2:
                nc.sync.dma_start(
                    _mm(x_sb, PROJ_DT),
                    _mm(xT_v[:, :, 512 * s : 512 * (s + 1)], PROJ_DT),
                )
            for w_sb, b_key, dst, scale in (
                (wq_sb, "bq_sb", qT_sb, HD ** -0.5),
                (wk_sb, "bk_sb", kT_sb, 1.0),
            ):
                for d4 in range(DT4):
                    ps = ppsum.tile([P, 512], F32, tag="pp", name=f"pp{s}_{d4}")
                    for kc in range(KC):
                        nc.tensor.matmul(
                            ps,
                            _mm(w_sb[:, kc, P * d4 : P * (d4 + 1)], PROJ_DT),
                            _mm(x_sb[:, kc, :], PROJ_DT),
                            start=(kc == 0),
                            stop=(kc == KC - 1),
                        )
                    nc.scalar.activation(
                        dst[:, d4, 512 * s : 512 * (s + 1)],
                        ps,
                        Act.Identity,
                        bias=st[b_key][:, d4 : d4 + 1],
                        scale=scale,
                    )
            for t2 in range(4):
                tt = 4 * s + t2
                ps = vpsum.tile([P, DG], F32, tag="ppv", name=f"ppv{tt}")
                for kc in range(KC):
                    nc.tensor.matmul(
                        ps,
                        _mm(x_sb[:, kc, P * t2 : P * (t2 + 1)], PROJ_DT),
                        _mm(wv_sb[:, kc, :], PROJ_DT),
                        start=(kc == 0),
                        stop=(kc == KC - 1),
                    )
                nc.vector.tensor_copy(
                    v_sb[:, tt].rearrange("p (h w) -> p h w", w=VW)[:, :, :HD],
                    ps.rearrange("p (h c) -> p h c", c=HD),
                )

def _phase_b(nc, tc, dbg, st, views):
    qT_sb, kT_sb, v_sb = st["qT_sb"], st["kT_sb"], st["v_sb"]
    ctxT_sb, wo_sb = st["ctxT_sb"], st["wo_sb"]
    wo_v, msk, mskd = views["wo_v"], views["msk"], views["mskd"]

    # ---------------- Phase B: sparse attention ----------------
    # Software-pipelined per head-half: ALL score matmuls + (merged, per-kv-
    # tile) exps are emitted first, then all ctx matmuls, so the in-order PE
    # queue never waits on the exp chain. Multiplicative masks run on the
    # (otherwise idle) GpSimd engine. The v ones-block makes the ctx matmul
    # emit softmax denominators replicated on psum rows 64:128, so
    # normalization is a plain DVE reciprocal+multiply (no PE broadcast).
    with (
        tc.tile_pool(name="apool", bufs=1) as apool,
        tc.tile_pool(name="tmppool", bufs=2) as tmppool,
        tc.tile_pool(name="atpool", bufs=12) as atpool,
        tc.tile_pool(name="spsum", bufs=3, space="PSUM") as spsum,
        tc.tile_pool(name="cpsum", bufs=1, space="PSUM") as cpsum,
    ):
        nc.sync.dma_start(_mm(wo_sb, OPROJ_DT), _mm(wo_v, OPROJ_DT))
        m_strict = apool.tile([P, P], ATTN_DT, name="m_strict")
        m_incl = apool.tile([P, P], ATTN_DT, name="m_incl")
        m_diag4 = apool.tile([P, 4 * P], ATTN_DT, name="m_diag4")
        nc.sync.dma_start(m_strict, msk[0])
        nc.sync.dma_start(m_incl, msk[1])
        nc.sync.dma_start(m_diag4, mskd)

        def scores_half(h, half):
            """Emit scores + merged exp + gpsimd mask; return ctx work list."""
            c, p0 = h // 2, HD * (h % 2)
            qh = qT_sb[p0 : p0 + HD, c, :]   # [64, 2048]
            kh = kT_sb[p0 : p0 + HD, c, :]
            mask = m_strict if half == 0 else m_incl
            work = []
            for j in range(NT):
                kv = kh[:, L + P * j : L + P * (j + 1)]                # [64, 128]
                vj = v_sb[:, NT + j, VW * h : VW * (h + 1)]            # [128, 128]
                sc = spsum.tile([P, L], F32, tag="sc", name=f"sc{h}_{j}_{half}")
                chunks = _chunks512(P * j, L)
                for a0, a1 in chunks:
                    nc.tensor.matmul(
                        sc[:, a0:a1],
                        kv,
                        qh[:, L * half + a0 : L * half + a1],
                        start=True,
                        stop=True,
                    )
                span = L - P * j
                at = atpool.tile([P, L], ATTN_DT, tag="at", name=f"at{h}_{j}_{half}")
                nc.scalar.activation(at[:, :span], sc[:, P * j :], Act.Exp)
                nc.gpsimd.tensor_mul(at[:, :P], at[:, :P], mask)
                for a0, a1 in chunks:
                    last = half == 1 and (
                        (a1 <= 512 and j == 3) or (a0 >= 512 and j == NT - 1)
                    )
                    work.append(
                        (vj, at[:, a0 - P * j : a1 - P * j], a0, a1, j == 0, last)
                    )
            if half == 0:
                # xt-xt block-diagonal tiles, 4 per psum/exp group
                for g in range(2):
                    scd = spsum.tile([P, L], F32, tag="sc", name=f"scd{h}_{g}")
                    for u in range(4):
                        i = 4 * g + u
                        nc.tensor.matmul(
                            scd[:, P * u : P * (u + 1)],
                            kh[:, P * i : P * (i + 1)],
                            qh[:, P * i : P * (i + 1)],
                            start=True,
                            stop=True,
                        )
                    atd = atpool.tile(
                        [P, L], ATTN_DT, tag="at", name=f"atd{h}_{g}"
                    )
                    nc.scalar.activation(atd[:, : 4 * P], scd[:, : 4 * P], Act.Exp)
                    nc.gpsimd.tensor_mul(atd[:, : 4 * P], atd[:, : 4 * P], m_diag4)
                    for u in range(4):
                        i = 4 * g + u
                        vd = v_sb[:, i, VW * h : VW * (h + 1)]
                        work.append(
                            (
                                vd,
                                atd[:, P * u : P * (u + 1)],
                                P * i,
                                P * (i + 1),
                                False,
                                i == 3 or i == NT - 1,
                            )
                        )
            return work

        def normalize(h, half, ctx):
            # rows 0:64 = sum(at*v); rows 64:128 = denom (replicated)
            c = h // 2
            recip = tmppool.tile([HD, L], F32, tag="recip", name=f"rc{h}_{half}")
            with nc.allow_low_precision(reason="deliberate f32r rounding"):
                nc.vector.reciprocal(_mm(recip, BCAST_DT), ctx[HD:P, :])
            if h % 2 == 0:
                nc.vector.tensor_mul(
                    _mm(ctxT_sb[:HD, c, L * half : L * (half + 1)], OPROJ_DT),
                    ctx[:HD, :],
                    recip,
                )
            else:
                cs = tmppool.tile([HD, L], F32, tag="cs", bufs=3, name=f"cs{h}_{half}")
                nc.vector.tensor_mul(_mm(cs, OPROJ_DT), ctx[:HD, :], recip)
                nc.sync.dma_start(
                    _mm(ctxT_sb[HD : 2 * HD, c, L * half : L * (half + 1)], OPROJ_DT),
                    _mm(cs, OPROJ_DT),
                )

        prev = None
        for h in range(HG):
            for half in range(2):
                work = scores_half(h, half)
                if prev is not None:
                    normalize(*prev)
                ctx = cpsum.tile([P, L], F32, tag="ctx", name=f"ctx{h}_{half}")
                for vj, at_sl, a0, a1, start, stop in work:
                    nc.tensor.matmul(ctx[:, a0:a1], vj, at_sl, start=start, stop=stop)
                prev = (h, half, ctx)
        normalize(*prev)

        if DBG:
            nc.sync.dma_start(dbg["dbg_qT"], qT_sb)
            nc.sync.dma_start(dbg["dbg_kT"], kT_sb)
            nc.sync.dma_start(dbg["dbg_v"], v_sb)
            nc.sync.dma_start(dbg["dbg_ctxT"], ctxT_sb)


def _phase_c(nc, tc, dbg, st, views):
    ctxT_sb, wo_sb = st["ctxT_sb"], st["wo_sb"]
    out = views["out"]

    # ---------------- Phase C: O-projection ----------------
    with (
        tc.tile_pool(name="opsum", bufs=6, space="PSUM") as opsum,
        tc.tile_pool(name="osbpool", bufs=6) as osbpool,
    ):
        for tt in range(T // P):
            for nk in range(2):
                ops = opsum.tile([P, 512], F32, tag="op", name=f"op{tt}_{nk}")
                for cc in range(DT4):
                    nc.tensor.matmul(
                        ops,
                        _mm(ctxT_sb[:, cc, P * tt : P * (tt + 1)], OPROJ_DT),
                        _mm(wo_sb[:, cc, 512 * nk : 512 * (nk + 1)], OPROJ_DT),
                        start=(cc == 0),
                        stop=(cc == DT4 - 1),
                    )
                osb = osbpool.tile([P, 512], F32, tag="osb", name=f"osb{tt}_{nk}")
                nc.vector.tensor_copy(osb, ops)
                nc.sync.dma_start(
                    out[P * tt : P * (tt + 1), 512 * nk : 512 * (nk + 1)], osb
                )


def _masks():
    """Multiplicative masks: msk[0] strict, msk[1] incl; mskd = 4 copies of
    the diag mask side by side (one gpsimd multiply covers a 4-tile group)."""
    q = np.arange(P)[None, :] // BS
    k = np.arange(P)[:, None] // BS
    m = np.zeros((2, P, P), np.float32)
    m[0] = (q > k).astype(np.float32)    # strict (xt q vs x0 k, same tile)
    m[1] = (q >= k).astype(np.float32)   # incl (x0 q vs x0 k, same tile)
    md = np.tile((q == k).astype(np.float32), (1, 4))  # diag, [P, 4P]
    return m, md


def kernel(x, Wq, bq, Wk, bk, Wv, bv, Wo, bo, block_size=4, **_):
    x = np.asarray(x, np.float32)
    Wq, bq = np.asarray(Wq, np.float32), np.asarray(bq, np.float32)
    Wk, bk = np.asarray(Wk, np.float32), np.asarray(bk, np.float32)
    Wv, bv = np.asarray(Wv, np.float32), np.asarray(bv, np.float32)
    Wo, bo = np.asarray(Wo, np.float32), np.asarray(bo, np.float32)

    if "nc" not in _CACHE:
        _CACHE["nc"] = _build()
    nc = _CACHE["nc"]

    masks, maskd = _masks()
    np_attn = mybir.dt.np(ATTN_DT)
    masks, maskd = masks.astype(np_attn), maskd.astype(np_attn)
    scale = HD ** -0.5
    in_maps = []
    for core in range(8):
        b, g = core // 2, core % 2
        cols = slice(DG * g, DG * (g + 1))
        in_maps.append(
            {
                "xT": np.ascontiguousarray(x[b].T),
                "wq": np.ascontiguousarray(Wq[:, cols]),
                "wk": np.ascontiguousarray(Wk[:, cols]),
                "wv": np.ascontiguousarray(Wv[:, cols]),
                "wo": np.ascontiguousarray(Wo[cols, :]),
                "bqs": np.ascontiguousarray(bq[cols]) * np.float32(scale),
                "bks": np.ascontiguousarray(bk[cols]),
                "msk": masks,
                "mskd": maskd,
            }
        )

    _CACHE["last_in_maps"] = in_maps
    last_err = None
    for _attempt in range(6):
        try:
            res = run_bass_kernel_spmd(nc, in_maps, core_ids=list(range(8)), trace=False)
            break
        except Exception as e:  # transient NRT device flakes
            last_err = e
            msg = str(e)
            if "UNRECOVERABLE" not in msg and "UNAVAILABLE" not in msg:
                raise
            import time as _time

            import jax as _jax

            _time.sleep(5 * (_attempt + 1))
            try:
                _jax.clear_backends()
            except Exception:
                pass
    else:
        raise last_err

    corr = (bv @ Wo + bo).astype(np.float32)  # softmax rows sum to 1
    out = np.empty((B, T, D), np.float32)
    for b in range(B):
        out[b] = res.results[2 * b]["out"] + res.results[2 * b + 1]["out"] + corr
    return out


if __name__ == "__main__":
    rng = np.random.default_rng(0)
    inputs = {
        "x": rng.standard_normal((B, T, D)).astype(np.float32),
        "Wq": (rng.standard_normal((D, D)) / 32).astype(np.float32),
        "bq": np.zeros(D, np.float32),
        "Wk": (rng.standard_normal((D, D)) / 32).astype(np.float32),
        "bk": np.zeros(D, np.float32),
        "Wv": (rng.standard_normal((D, D)) / 32).astype(np.float32),
        "bv": np.zeros(D, np.float32),
        "Wo": (rng.standard_normal((D, D)) / 32).astype(np.float32),
        "bo": np.zeros(D, np.float32),
    }
    o = kernel(**inputs)
    print("ran", o.shape, o.dtype, float(np.abs(o).max()))

